# revision 3
# baseline (speedup 1.0000x reference)
"""KAST scatter-memory kernel for Trainium2 (8 NeuronCores, data-parallel over batch).

Per core: one batch element, 15 sequential steps.

Host precomputes (numpy, inside kernel()):
  kt   = k transposed to [seq, ck, hw]   (no on-device PE transposes)
  g    = sigmoid(attention)              (no on-device sigmoid / ACT table swaps)
  v4   = [v | 1] ones-padded             (denominator column ready)

Device, per step i:
  m_kT = m_kT + G*(kT_i - m_kT)          (Pool engine, ping-pong buffers)
  L    = kT_i^T/m_kT^T @ kT_{i+1}        ([kk, q] fp32r matmuls, [128,1024] psum)
  E    = exp(L - 60)                     (one 1024-wide ACT exp per tile)
  rec  = E^T(stationary) @ [pv|1]/[mv|1] (tiny [128,4]-output matmuls, natural layout)
  rec  = 0.9*Nk/Dk + 0.1*Nm/Dm          (DVE reciprocal/blend)
  pv   = mask_i ? v_i : rec
"""
import sys

sys.path.insert(0, "/opt/trn_rl_repo")

import numpy as np

import concourse.bass as bass
import concourse.tile as tile
from concourse import bacc, mybir
from concourse.bass_utils import run_bass_kernel_spmd

F32 = mybir.dt.float32
F32R = mybir.dt.float32r
AF = mybir.ActivationFunctionType

BS, SEQ, H, W, CK = 8, 16, 32, 32, 256
HW = H * W          # 1024
CV = 3
NT = HW // 128      # 8 hw tiles
NC2 = CK // 128     # 2 ck chunks
SHIFT = 60.0        # exp(logit - SHIFT); logits empirically <= 136, rowmax >= 23
COEF = 0.1

_CACHE = {}


def _r(x):
    return x.bitcast(F32R)


def build_program():
    nc = bacc.Bacc("TRN2", target_bir_lowering=False, debug=False, num_devices=8)

    kt_d = nc.dram_tensor("kt", [SEQ, CK, HW], F32R, kind="ExternalInput")
    v4_d = nc.dram_tensor("v4", [SEQ, HW, 4], F32R, kind="ExternalInput")
    g_d = nc.dram_tensor("g", [SEQ, HW], F32, kind="ExternalInput")
    m_d = nc.dram_tensor("maskf", [1, SEQ], F32, kind="ExternalInput")
    o_d = nc.dram_tensor("out_v", [SEQ - 1, HW, CV], F32, kind="ExternalOutput")

    with tile.TileContext(nc) as tc:
        with (
            tc.tile_pool(name="persist", bufs=1) as P1,
            tc.tile_pool(name="kt", bufs=4) as PKT,
            tc.tile_pool(name="gb", bufs=3) as PG,
            tc.tile_pool(name="tmp", bufs=2) as PT,
            tc.tile_pool(name="ek", bufs=10) as PEK,
            tc.tile_pool(name="em", bufs=10) as PEM,
            tc.tile_pool(name="small", bufs=3) as PSM,
            tc.tile_pool(name="psA", bufs=3, space="PSUM") as PSA,
            tc.tile_pool(name="psC", bufs=2, space="PSUM") as PSC,
        ):
            negC = P1.tile([128, 1], F32)
            nc.vector.memset(negC, -SHIFT)

            # persistent state: m_kT ping-pong [128, (c w)] = [ck-part, 2 x hw]
            m_kT = [P1.tile([128, NC2 * HW], F32, tag=f"mkT{j}", name=f"mkT{j}") for j in range(2)]
            nc.vector.memset(m_kT[0], 0.0)
            mv1 = P1.tile([128, 4 * NT], F32, tag="mv1")
            nc.vector.memset(mv1, 0.0)
            nc.vector.memset(mv1[:, 3 : 4 * NT : 4], 1.0)

            def load_v1(i):
                t = PSM.tile([128, NT, 4], F32R, tag="v1")
                nc.gpsimd.dma_start(
                    out=t, in_=v4_d[i].rearrange("(t p) c -> p t c", p=128)
                )
                return t.rearrange("p t c -> p (t c)")

            def load_kT(i):
                kT = PKT.tile([128, NC2, HW], F32R, tag="kT", name=f"kT{i}")
                nc.sync.dma_start(
                    out=kT, in_=kt_d[i].rearrange("(c p) w -> p c w", p=128)
                )
                return kT.rearrange("p c w -> p (c w)")

            # prologue
            pv1 = load_v1(0)
            kT_i = load_kT(0)
            kT_n = load_kT(1)

            for i in range(SEQ - 1):
                kT_n2 = load_kT(i + 2) if i + 2 <= SEQ - 1 else None

                # --- gate G_i = sigmoid(att[i]) (precomputed host-side) broadcast
                G = PG.tile([128, HW], F32, tag="G")
                nc.gpsimd.dma_start(
                    out=G, in_=g_d[i : i + 1, :].partition_broadcast(128)
                )
                # natural-layout gate for m_v update, replicated x4
                gnat = PSM.tile([128, NT], F32, tag="gnat")
                nc.gpsimd.dma_start(
                    out=gnat, in_=g_d[i].rearrange("(t p) -> p t", p=128)
                )
                gb32 = PSM.tile([128, NT, 4], F32, tag="gb32")
                nc.vector.tensor_copy(
                    out=gb32, in_=gnat.unsqueeze(-1).broadcast_to([128, NT, 4])
                )
                gb32 = gb32.rearrange("p t c -> p (t c)")

                # --- m_kT EMA on Pool engine (ping-pong): mnew = mcur + G*(kT_i - mcur)
                mcur, mnew = m_kT[i % 2], m_kT[(i + 1) % 2]
                for c in range(NC2):
                    sl = slice(c * HW, (c + 1) * HW)
                    tmp = PT.tile([128, HW], F32, tag=f"tmpk{c}")
                    nc.gpsimd.tensor_sub(tmp, kT_i[:, sl], mcur[:, sl])
                    nc.gpsimd.tensor_mul(tmp, tmp, G)
                    nc.gpsimd.tensor_add(_r(mnew[:, sl]), mcur[:, sl], tmp)

                # --- m_v EMA: mv1 += gb32 * (pv1 - mv1)  (ones col stays 1)
                tmpv = PSM.tile([128, 4 * NT], F32, tag="tmpv")
                nc.vector.tensor_sub(tmpv, pv1, mv1)
                nc.vector.tensor_mul(tmpv, tmpv, gb32)
                nc.vector.tensor_add(_r(mv1), mv1, tmpv)

                # --- logits + exp for both sims, [kk part, q free]
                E_k = [None] * NT
                E_m = [None] * NT
                for lhs, E, pool, nm in (
                    (kT_i, E_k, PEK, "ek"),
                    (mnew, E_m, PEM, "em"),
                ):
                    for t in range(NT):
                        ps = PSA.tile([128, HW], F32, tag="big", name=f"ps{nm}{i}_{t}")
                        for half in range(2):
                            for c in range(NC2):
                                nc.tensor.matmul(
                                    ps[:, half * 512 : (half + 1) * 512],
                                    _r(lhs[:, c * HW + t * 128 : c * HW + (t + 1) * 128]),
                                    _r(kT_n[:, c * HW + half * 512 : c * HW + (half + 1) * 512]),
                                    start=(c == 0),
                                    stop=(c == NC2 - 1),
                                )
                        E[t] = pool.tile([128, HW], F32, tag=nm, name=f"{nm}{i}_{t}")
                        nc.scalar.activation(
                            _r(E[t]), ps, AF.Exp, bias=negC[:, 0:1]
                        )

                # --- rec (natural layout): psN[:, 0:32] = E_k^T @ [pv|1], [:, 32:64] = E_m^T @ [mv|1]
                psN = PSC.tile([128, 64], F32, tag="recps", name=f"recps{i}")
                for s, (E, rhs1) in enumerate(((E_k, pv1), (E_m, mv1))):
                    for q in range(NT):
                        out_sl = psN[:, s * 32 + q * 4 : s * 32 + (q + 1) * 4]
                        for c in range(NT):
                            nc.tensor.matmul(
                                out_sl,
                                _r(E[c][:, q * 128 : (q + 1) * 128]),
                                _r(rhs1[:, c * 4 : (c + 1) * 4]),
                                start=(c == 0),
                                stop=(c == NT - 1),
                            )

                # --- rec = 0.9*Nk/Dk + 0.1*Nm/Dm
                Nnat = PSM.tile([128, 64], F32, tag="Nnat")
                nc.vector.tensor_copy(out=Nnat, in_=psN)
                rD = PSM.tile([128, 16], F32, tag="rD")
                nc.vector.reciprocal(rD, Nnat[:, 3:64:4])
                nc.vector.tensor_scalar_mul(rD[:, 0:8], rD[:, 0:8], 1.0 - COEF)
                nc.vector.tensor_scalar_mul(rD[:, 8:16], rD[:, 8:16], COEF)
                rDe = PSM.tile([128, 16, 4], F32, tag="rDe")
                nc.vector.tensor_copy(
                    out=rDe, in_=rD.unsqueeze(-1).broadcast_to([128, 16, 4])
                )
                rDe = rDe.rearrange("p t c -> p (t c)")
                Ns = PSM.tile([128, 64], F32, tag="Ns")
                nc.vector.tensor_mul(Ns, Nnat, rDe)
                rec = PSM.tile([128, 32], F32, tag="rec")
                nc.vector.tensor_add(rec, Ns[:, 0:32], Ns[:, 32:64])

                # --- write out_v[i] (pre-blend reconstruction)
                nc.sync.dma_start(
                    out=o_d[i].rearrange("(t p) c -> p t c", p=128),
                    in_=rec.rearrange("p (t c) -> p t c", c=4)[:, :, 0:CV],
                )

                # --- pv_next = rec + mask_i * (v1_i - rec)
                if i < SEQ - 2:
                    Msc = PSM.tile([128, 1], F32, tag="Msc")
                    nc.gpsimd.dma_start(
                        out=Msc, in_=m_d[0:1, i : i + 1].partition_broadcast(128)
                    )
                    v1 = load_v1(i)
                    diff = PSM.tile([128, 32], F32, tag="diff")
                    nc.vector.tensor_sub(diff, v1, rec)
                    nc.vector.tensor_scalar_mul(diff, diff, Msc[:, 0:1])
                    pv1_new = PSM.tile([128, 32], F32, tag="pv1")
                    nc.vector.tensor_add(_r(pv1_new), rec, diff)
                    pv1 = pv1_new
                    kT_i = kT_n
                    kT_n = kT_n2

    nc.compile()
    return nc


def kernel(k, v, attention, seq_mask):
    k = np.asarray(k, dtype=np.float32)
    v = np.asarray(v, dtype=np.float32)
    attention = np.asarray(attention, dtype=np.float32)
    seq_mask = np.asarray(seq_mask)

    if "nc" not in _CACHE:
        _CACHE["nc"] = build_program()
    nc = _CACHE["nc"]

    # host-side preprocessing
    kt = np.ascontiguousarray(
        k.reshape(BS, SEQ, HW, CK).transpose(0, 1, 3, 2)
    )  # [bs, seq, ck, hw]
    v4 = np.ones((BS, SEQ, HW, 4), np.float32)
    v4[:, :, :, :CV] = v.reshape(BS, SEQ, HW, CV)
    g = 1.0 / (1.0 + np.exp(-attention.reshape(BS, SEQ, HW)))
    maskf = seq_mask.astype(np.float32)

    in_maps = []
    for b in range(BS):
        in_maps.append(
            {
                "kt": kt[b],
                "v4": v4[b],
                "g": np.ascontiguousarray(g[b]),
                "maskf": np.ascontiguousarray(maskf[b : b + 1]),
            }
        )
    res = run_bass_kernel_spmd(nc, in_maps, list(range(BS)))
    out_v = np.stack([res.results[b]["out_v"] for b in range(BS)]).reshape(
        BS, SEQ - 1, H, W, CV
    )
    gt = v[:, 1:].reshape(BS, SEQ - 1, H, W, CV)
    return out_v, gt


# revision 5
# speedup vs baseline: 1.0409x; 1.0409x over previous
"""KAST scatter-memory kernel for Trainium2 (8 NeuronCores, data-parallel over batch).

Per core: one batch element, 15 sequential steps.

Host precomputes (numpy, inside kernel()):
  kt   = k transposed to [seq, ck, hw]   (no on-device PE transposes)
  g    = sigmoid(attention)              (no on-device sigmoid / ACT table swaps)
  v4   = [v | 1] ones-padded, partition-major  (denominator column ready)
  g4   = gate natural-layout replicated x4, partition-major

Device, per step i:
  m_kT = m_kT + G*(kT_i - m_kT)          (DVE, ping-pong buffers)
  L    = kT_i^T/m_kT^T @ kT_{i+1}        ([kk, q] fp32r matmuls, [128,1024] psum)
  E    = exp(L - 60)                     (one 1024-wide ACT exp per tile)
  rec  = E^T(stationary) @ [pv|1]/[mv|1] (tiny [128,4]-output matmuls, natural layout)
  rec  = 0.9*Nk/Dk + 0.1*Nm/Dm          (DVE reciprocal/blend)
  pv   = mask_i ? v_i : rec

All gate/v/mask tensors are preloaded in a handful of bulk DMAs; the steady
state issues only the kT frame load and the out_v store.
"""
import sys

sys.path.insert(0, "/opt/trn_rl_repo")

import numpy as np

import concourse.bass as bass
import concourse.tile as tile
from concourse import bacc, mybir
from concourse.bass_utils import run_bass_kernel_spmd

F32 = mybir.dt.float32
F32R = mybir.dt.float32r
BF16 = mybir.dt.bfloat16
AF = mybir.ActivationFunctionType

BS, SEQ, H, W, CK = 8, 16, 32, 32, 256
HW = H * W          # 1024
CV = 3
NT = HW // 128      # 8 hw tiles
NC2 = CK // 128     # 2 ck chunks
SHIFT = 60.0        # exp(logit - SHIFT); logits empirically <= 136, rowmax >= 23
COEF = 0.1

_CACHE = {}


def _r(x):
    return x.bitcast(F32R)


def build_program():
    nc = bacc.Bacc("TRN2", target_bir_lowering=False, debug=False, num_devices=8)

    kt_d = nc.dram_tensor("kt", [SEQ, CK, HW], F32R, kind="ExternalInput")
    v4_d = nc.dram_tensor("v4h", [128, SEQ, NT * 4], F32R, kind="ExternalInput")
    g_d = nc.dram_tensor("gflat", [1, SEQ * HW], BF16, kind="ExternalInput")
    g4_d = nc.dram_tensor("g4h", [128, SEQ, NT * 4], F32, kind="ExternalInput")
    m_d = nc.dram_tensor("maskf", [1, SEQ], F32, kind="ExternalInput")
    o_d = nc.dram_tensor("out_v", [SEQ - 1, HW, CV], F32, kind="ExternalOutput")

    with tile.TileContext(nc) as tc:
        with (
            tc.tile_pool(name="persist", bufs=1) as P1,
            tc.tile_pool(name="kt", bufs=4) as PKT,
            tc.tile_pool(name="tmp", bufs=2) as PT,
            tc.tile_pool(name="ek", bufs=10) as PEK,
            tc.tile_pool(name="em", bufs=10) as PEM,
            tc.tile_pool(name="small", bufs=3) as PSM,
            tc.tile_pool(name="psA", bufs=3, space="PSUM") as PSA,
            tc.tile_pool(name="psC", bufs=2, space="PSUM") as PSC,
        ):
            negC = P1.tile([128, 1], F32)
            nc.vector.memset(negC, -SHIFT)

            # ---- bulk preloads (steady state has no small DMAs)
            # broadcast gate, bf16, one row per step: [128, seq-1, hw]
            Gall = P1.tile([128, SEQ - 1, HW], BF16, tag="Gall")
            for lo, hi in ((0, 4), (4, SEQ - 1)):
                nc.gpsimd.dma_start(
                    out=Gall[:, lo:hi, :],
                    in_=g_d[0:1, lo * HW : hi * HW].partition_broadcast(128),
                )
            # [v|1] per step, natural layout: [128, seq, 8, 4]
            Vall = P1.tile([128, SEQ, NT * 4], F32R, tag="Vall")
            nc.sync.dma_start(out=Vall, in_=v4_d[:, :, :])
            # natural gate replicated x4: [128, seq, 32]
            G4all = P1.tile([128, SEQ, NT * 4], F32, tag="G4all")
            nc.sync.dma_start(out=G4all, in_=g4_d[:, :, :])
            # mask row: [128, seq]
            Mall = P1.tile([128, SEQ], F32, tag="Mall")
            nc.gpsimd.dma_start(out=Mall, in_=m_d[0:1, :].partition_broadcast(128))

            # persistent state: m_kT ping-pong [128, (c w)] = [ck-part, 2 x hw]
            m_kT = [P1.tile([128, NC2 * HW], F32, tag=f"mkT{j}", name=f"mkT{j}") for j in range(2)]
            nc.vector.memset(m_kT[0], 0.0)
            mv1 = P1.tile([128, 4 * NT], F32, tag="mv1")
            nc.vector.memset(mv1, 0.0)
            nc.vector.memset(mv1[:, 3 : 4 * NT : 4], 1.0)

            def load_kT(i):
                kT = PKT.tile([128, NC2, HW], F32R, tag="kT", name=f"kT{i}")
                nc.sync.dma_start(
                    out=kT, in_=kt_d[i].rearrange("(c p) w -> p c w", p=128)
                )
                return kT.rearrange("p c w -> p (c w)")

            # prologue
            pv1 = Vall[:, 0, :]
            kT_i = load_kT(0)
            kT_n = load_kT(1)

            for i in range(SEQ - 1):
                kT_n2 = load_kT(i + 2) if i + 2 <= SEQ - 1 else None
                G = Gall[:, i, :]
                gb32 = G4all[:, i, :]

                # --- m_kT EMA on DVE (ping-pong): mnew = mcur + G*(kT_i - mcur)
                mcur, mnew = m_kT[i % 2], m_kT[(i + 1) % 2]
                for c in range(NC2):
                    sl = slice(c * HW, (c + 1) * HW)
                    tmp = PT.tile([128, HW], F32, tag=f"tmpk{c}")
                    nc.vector.tensor_sub(tmp, kT_i[:, sl].bitcast(F32), mcur[:, sl])
                    nc.vector.tensor_mul(tmp, tmp, G)
                    nc.vector.tensor_add(_r(mnew[:, sl]), mcur[:, sl], tmp)

                # --- m_v EMA: mv1 += gb32 * (pv1 - mv1)  (ones col stays 1)
                tmpv = PSM.tile([128, 4 * NT], F32, tag="tmpv")
                nc.vector.tensor_sub(tmpv, pv1.bitcast(F32), mv1)
                nc.vector.tensor_mul(tmpv, tmpv, gb32)
                nc.vector.tensor_add(_r(mv1), mv1, tmpv)

                # --- logits + exp for both sims, [kk part, q free]
                E_k = [None] * NT
                E_m = [None] * NT
                for lhs, E, pool, nm in (
                    (kT_i, E_k, PEK, "ek"),
                    (mnew, E_m, PEM, "em"),
                ):
                    for t in range(NT):
                        ps = PSA.tile([128, HW], F32, tag="big", name=f"ps{nm}{i}_{t}")
                        for half in range(2):
                            for c in range(NC2):
                                nc.tensor.matmul(
                                    ps[:, half * 512 : (half + 1) * 512],
                                    _r(lhs[:, c * HW + t * 128 : c * HW + (t + 1) * 128]),
                                    _r(kT_n[:, c * HW + half * 512 : c * HW + (half + 1) * 512]),
                                    start=(c == 0),
                                    stop=(c == NC2 - 1),
                                )
                        E[t] = pool.tile([128, HW], F32, tag=nm, name=f"{nm}{i}_{t}")
                        nc.scalar.activation(
                            _r(E[t]), ps, AF.Exp, bias=negC[:, 0:1]
                        )

                # --- rec (natural layout): psN[:, 0:32] = E_k^T @ [pv|1], [:, 32:64] = E_m^T @ [mv|1]
                psN = PSC.tile([128, 64], F32, tag="recps", name=f"recps{i}")
                for s, (E, rhs1) in enumerate(((E_k, pv1), (E_m, mv1))):
                    for q in range(NT):
                        out_sl = psN[:, s * 32 + q * 4 : s * 32 + (q + 1) * 4]
                        for c in range(NT):
                            nc.tensor.matmul(
                                out_sl,
                                _r(E[c][:, q * 128 : (q + 1) * 128]),
                                _r(rhs1[:, c * 4 : (c + 1) * 4]),
                                start=(c == 0),
                                stop=(c == NT - 1),
                            )

                # --- rec = 0.9*Nk/Dk + 0.1*Nm/Dm
                Nnat = PSM.tile([128, 64], F32, tag="Nnat")
                nc.vector.tensor_copy(out=Nnat, in_=psN)
                rD = PSM.tile([128, 16], F32, tag="rD")
                nc.vector.reciprocal(rD, Nnat[:, 3:64:4])
                nc.vector.tensor_scalar_mul(rD[:, 0:8], rD[:, 0:8], 1.0 - COEF)
                nc.vector.tensor_scalar_mul(rD[:, 8:16], rD[:, 8:16], COEF)
                rDe = PSM.tile([128, 16, 4], F32, tag="rDe")
                nc.vector.tensor_copy(
                    out=rDe, in_=rD.unsqueeze(-1).broadcast_to([128, 16, 4])
                )
                rDe = rDe.rearrange("p t c -> p (t c)")
                Ns = PSM.tile([128, 64], F32, tag="Ns")
                nc.vector.tensor_mul(Ns, Nnat, rDe)
                rec = PSM.tile([128, 32], F32, tag="rec")
                nc.vector.tensor_add(rec, Ns[:, 0:32], Ns[:, 32:64])

                # --- write out_v[i] (pre-blend reconstruction)
                nc.sync.dma_start(
                    out=o_d[i].rearrange("(t p) c -> p t c", p=128),
                    in_=rec.rearrange("p (t c) -> p t c", c=4)[:, :, 0:CV],
                )

                # --- pv_next = rec + mask_i * (v1_i - rec)
                if i < SEQ - 2:
                    v1 = Vall[:, i, :]
                    diff = PSM.tile([128, 32], F32, tag="diff")
                    nc.vector.tensor_sub(diff, v1.bitcast(F32), rec)
                    nc.vector.tensor_scalar_mul(diff, diff, Mall[:, i : i + 1])
                    pv1_new = PSM.tile([128, 32], F32, tag="pv1")
                    nc.vector.tensor_add(_r(pv1_new), rec, diff)
                    pv1 = pv1_new
                    kT_i = kT_n
                    kT_n = kT_n2

    nc.compile()
    return nc


def _prep_inputs(k, v, attention, seq_mask):
    kt = np.ascontiguousarray(
        k.reshape(BS, SEQ, HW, CK).transpose(0, 1, 3, 2)
    )  # [bs, seq, ck, hw]
    v4 = np.ones((BS, SEQ, HW, 4), np.float32)
    v4[:, :, :, :CV] = v.reshape(BS, SEQ, HW, CV)
    # partition-major: [bs, 128, seq, 8, 4]
    v4h = np.ascontiguousarray(
        v4.reshape(BS, SEQ, NT, 128, 4).transpose(0, 3, 1, 2, 4).reshape(
            BS, 128, SEQ, NT * 4
        )
    )
    import ml_dtypes

    g = (1.0 / (1.0 + np.exp(-attention.reshape(BS, SEQ, HW)))).astype(np.float32)
    gflat = np.ascontiguousarray(
        g.astype(ml_dtypes.bfloat16).reshape(BS, 1, SEQ * HW)
    )
    g4h = np.ascontiguousarray(
        np.repeat(
            g.reshape(BS, SEQ, NT, 128)[:, :, :, :, None], 4, axis=4
        ).transpose(0, 3, 1, 2, 4).reshape(BS, 128, SEQ, NT * 4)
    )
    maskf = seq_mask.astype(np.float32)
    return kt, v4h, gflat, g4h, maskf


def kernel(k, v, attention, seq_mask):
    k = np.asarray(k, dtype=np.float32)
    v = np.asarray(v, dtype=np.float32)
    attention = np.asarray(attention, dtype=np.float32)
    seq_mask = np.asarray(seq_mask)

    if "nc" not in _CACHE:
        _CACHE["nc"] = build_program()
    nc = _CACHE["nc"]

    kt, v4h, gflat, g4h, maskf = _prep_inputs(k, v, attention, seq_mask)

    in_maps = []
    for b in range(BS):
        in_maps.append(
            {
                "kt": kt[b],
                "v4h": v4h[b],
                "gflat": gflat[b],
                "g4h": g4h[b],
                "maskf": np.ascontiguousarray(maskf[b : b + 1]),
            }
        )
    res = run_bass_kernel_spmd(nc, in_maps, list(range(BS)))
    out_v = np.stack([res.results[b]["out_v"] for b in range(BS)]).reshape(
        BS, SEQ - 1, H, W, CV
    )
    gt = v[:, 1:].reshape(BS, SEQ - 1, H, W, CV)
    return out_v, gt


# revision 6
# speedup vs baseline: 1.0852x; 1.0425x over previous
"""KAST scatter-memory kernel for Trainium2 (8 NeuronCores, data-parallel over batch).

Per core: one batch element, 15 sequential steps.

Host precomputes (numpy, inside kernel()):
  kt   = k transposed to [seq, ck, hw]   (no on-device PE transposes)
  g    = sigmoid(attention)              (no on-device sigmoid / ACT table swaps)
  v4   = [v | 1] ones-padded, partition-major  (denominator column ready)
  g4   = gate natural-layout replicated x4, partition-major

Device, per step i:
  m_kT = m_kT + G*(kT_i - m_kT)          (DVE, ping-pong buffers)
  L    = kT_i^T/m_kT^T @ kT_{i+1}        ([kk, q] fp32r matmuls, [128,1024] psum)
  E    = exp(L - 60)                     (one 1024-wide ACT exp per tile)
  rec  = E^T(stationary) @ [pv|1]/[mv|1] (tiny [128,4]-output matmuls, natural layout)
  rec  = 0.9*Nk/Dk + 0.1*Nm/Dm          (DVE reciprocal/blend)
  pv   = mask_i ? v_i : rec

All gate/v/mask tensors are preloaded in a handful of bulk DMAs; the steady
state issues only the kT frame load and the out_v store.
"""
import sys

sys.path.insert(0, "/opt/trn_rl_repo")

import numpy as np

import concourse.bass as bass
import concourse.tile as tile
from concourse import bacc, mybir
from concourse.bass_utils import run_bass_kernel_spmd

F32 = mybir.dt.float32
F32R = mybir.dt.float32r
BF16 = mybir.dt.bfloat16
AF = mybir.ActivationFunctionType

BS, SEQ, H, W, CK = 8, 16, 32, 32, 256
HW = H * W          # 1024
CV = 3
NT = HW // 128      # 8 hw tiles
NC2 = CK // 128     # 2 ck chunks
SHIFT = 60.0        # exp(logit - SHIFT); logits empirically <= 136, rowmax >= 23
COEF = 0.1

_CACHE = {}


def _r(x):
    return x.bitcast(F32R)


def build_program():
    nc = bacc.Bacc("TRN2", target_bir_lowering=False, debug=False, num_devices=8)

    kt_d = nc.dram_tensor("kt", [SEQ, CK, HW], F32R, kind="ExternalInput")
    v4_d = nc.dram_tensor("v4h", [128, SEQ, NT * 4], F32R, kind="ExternalInput")
    g_d = nc.dram_tensor("gflat", [1, SEQ * HW], BF16, kind="ExternalInput")
    g4_d = nc.dram_tensor("g4h", [128, SEQ, NT * 4], F32, kind="ExternalInput")
    m_d = nc.dram_tensor("maskf", [1, SEQ], F32, kind="ExternalInput")
    o_d = nc.dram_tensor("out_v", [SEQ - 1, HW, CV], F32, kind="ExternalOutput")

    with tile.TileContext(nc) as tc:
        with (
            tc.tile_pool(name="persist", bufs=1) as P1,
            tc.tile_pool(name="kt", bufs=4) as PKT,
            tc.tile_pool(name="tmp", bufs=2) as PT,
            tc.tile_pool(name="gb", bufs=3) as PG,
            tc.tile_pool(name="ek", bufs=10) as PEK,
            tc.tile_pool(name="em", bufs=10) as PEM,
            tc.tile_pool(name="small", bufs=3) as PSM,
            tc.tile_pool(name="psA", bufs=3, space="PSUM") as PSA,
            tc.tile_pool(name="psC", bufs=2, space="PSUM") as PSC,
        ):
            negC = P1.tile([128, 1], F32)
            nc.vector.memset(negC, -SHIFT)

            # persistent state: m_kT ping-pong [128, (c w)] = [ck-part, 2 x hw]
            m_kT = [P1.tile([128, NC2 * HW], F32, tag=f"mkT{j}", name=f"mkT{j}") for j in range(2)]
            nc.vector.memset(m_kT[0], 0.0)
            mv1 = P1.tile([128, 4 * NT], F32, tag="mv1")
            nc.vector.memset(mv1, 0.0)
            nc.vector.memset(mv1[:, 3 : 4 * NT : 4], 1.0)

            def load_kT(i):
                kT = PKT.tile([128, NC2, HW], F32R, tag="kT", name=f"kT{i}")
                nc.sync.dma_start(
                    out=kT, in_=kt_d[i].rearrange("(c p) w -> p c w", p=128)
                )
                return kT.rearrange("p c w -> p (c w)")

            def load_G(i):
                Gt = PG.tile([128, HW], BF16, tag="G", name=f"G{i}")
                nc.gpsimd.dma_start(
                    out=Gt, in_=g_d[0:1, i * HW : (i + 1) * HW].partition_broadcast(128)
                )
                return Gt

            # prologue: kT frames first -- the serial DMA stream must deliver
            # kT1 before anything bulky, since the first matmul waits on it
            kT_i = load_kT(0)
            kT_n = load_kT(1)
            G_rows = [load_G(0), load_G(1)]
            # [v|1] per step, natural layout: [128, seq, 8*4]
            Vall = P1.tile([128, SEQ, NT * 4], F32R, tag="Vall")
            nc.sync.dma_start(out=Vall, in_=v4_d[:, :, :])
            # natural gate replicated x4: [128, seq, 32]
            G4all = P1.tile([128, SEQ, NT * 4], F32, tag="G4all")
            nc.sync.dma_start(out=G4all, in_=g4_d[:, :, :])
            # mask row: [128, seq]
            Mall = P1.tile([128, SEQ], F32, tag="Mall")
            nc.gpsimd.dma_start(out=Mall, in_=m_d[0:1, :].partition_broadcast(128))
            pv1 = Vall[:, 0, :]

            for i in range(SEQ - 1):
                kT_n2 = load_kT(i + 2) if i + 2 <= SEQ - 1 else None
                if i + 2 <= SEQ - 2:
                    G_rows.append(load_G(i + 2))
                G = G_rows[i]
                gb32 = G4all[:, i, :]

                # --- m_kT EMA on DVE (ping-pong): mnew = mcur + G*(kT_i - mcur)
                mcur, mnew = m_kT[i % 2], m_kT[(i + 1) % 2]
                for c in range(NC2):
                    sl = slice(c * HW, (c + 1) * HW)
                    tmp = PT.tile([128, HW], F32, tag=f"tmpk{c}")
                    nc.vector.tensor_sub(tmp, kT_i[:, sl].bitcast(F32), mcur[:, sl])
                    nc.vector.tensor_mul(tmp, tmp, G)
                    nc.vector.tensor_add(_r(mnew[:, sl]), mcur[:, sl], tmp)

                # --- m_v EMA: mv1 += gb32 * (pv1 - mv1)  (ones col stays 1)
                tmpv = PSM.tile([128, 4 * NT], F32, tag="tmpv")
                nc.vector.tensor_sub(tmpv, pv1.bitcast(F32), mv1)
                nc.vector.tensor_mul(tmpv, tmpv, gb32)
                nc.vector.tensor_add(_r(mv1), mv1, tmpv)

                # --- logits + exp for both sims, [kk part, q free]
                E_k = [None] * NT
                E_m = [None] * NT
                for lhs, E, pool, nm in (
                    (kT_i, E_k, PEK, "ek"),
                    (mnew, E_m, PEM, "em"),
                ):
                    for t in range(NT):
                        ps = PSA.tile([128, HW], F32, tag="big", name=f"ps{nm}{i}_{t}")
                        for half in range(2):
                            for c in range(NC2):
                                nc.tensor.matmul(
                                    ps[:, half * 512 : (half + 1) * 512],
                                    _r(lhs[:, c * HW + t * 128 : c * HW + (t + 1) * 128]),
                                    _r(kT_n[:, c * HW + half * 512 : c * HW + (half + 1) * 512]),
                                    start=(c == 0),
                                    stop=(c == NC2 - 1),
                                )
                        E[t] = pool.tile([128, HW], F32, tag=nm, name=f"{nm}{i}_{t}")
                        nc.scalar.activation(
                            _r(E[t]), ps, AF.Exp, bias=negC[:, 0:1]
                        )

                # --- rec (natural layout): psN[:, 0:32] = E_k^T @ [pv|1], [:, 32:64] = E_m^T @ [mv|1]
                psN = PSC.tile([128, 64], F32, tag="recps", name=f"recps{i}")
                for s, (E, rhs1) in enumerate(((E_k, pv1), (E_m, mv1))):
                    for q in range(NT):
                        out_sl = psN[:, s * 32 + q * 4 : s * 32 + (q + 1) * 4]
                        for c in range(NT):
                            nc.tensor.matmul(
                                out_sl,
                                _r(E[c][:, q * 128 : (q + 1) * 128]),
                                _r(rhs1[:, c * 4 : (c + 1) * 4]),
                                start=(c == 0),
                                stop=(c == NT - 1),
                            )

                # --- rec = 0.9*Nk/Dk + 0.1*Nm/Dm
                Nnat = PSM.tile([128, 64], F32, tag="Nnat")
                nc.vector.tensor_copy(out=Nnat, in_=psN)
                rD = PSM.tile([128, 16], F32, tag="rD")
                nc.vector.reciprocal(rD, Nnat[:, 3:64:4])
                nc.vector.tensor_scalar_mul(rD[:, 0:8], rD[:, 0:8], 1.0 - COEF)
                nc.vector.tensor_scalar_mul(rD[:, 8:16], rD[:, 8:16], COEF)
                rDe = PSM.tile([128, 16, 4], F32, tag="rDe")
                nc.vector.tensor_copy(
                    out=rDe, in_=rD.unsqueeze(-1).broadcast_to([128, 16, 4])
                )
                rDe = rDe.rearrange("p t c -> p (t c)")
                Ns = PSM.tile([128, 64], F32, tag="Ns")
                nc.vector.tensor_mul(Ns, Nnat, rDe)
                rec = PSM.tile([128, 32], F32, tag="rec")
                nc.vector.tensor_add(rec, Ns[:, 0:32], Ns[:, 32:64])

                # --- write out_v[i] (pre-blend reconstruction)
                nc.sync.dma_start(
                    out=o_d[i].rearrange("(t p) c -> p t c", p=128),
                    in_=rec.rearrange("p (t c) -> p t c", c=4)[:, :, 0:CV],
                )

                # --- pv_next = rec + mask_i * (v1_i - rec)
                if i < SEQ - 2:
                    v1 = Vall[:, i, :]
                    diff = PSM.tile([128, 32], F32, tag="diff")
                    nc.vector.tensor_sub(diff, v1.bitcast(F32), rec)
                    nc.vector.tensor_scalar_mul(diff, diff, Mall[:, i : i + 1])
                    pv1_new = PSM.tile([128, 32], F32, tag="pv1")
                    nc.vector.tensor_add(_r(pv1_new), rec, diff)
                    pv1 = pv1_new
                    kT_i = kT_n
                    kT_n = kT_n2

    nc.compile()
    return nc


def _prep_inputs(k, v, attention, seq_mask):
    kt = np.ascontiguousarray(
        k.reshape(BS, SEQ, HW, CK).transpose(0, 1, 3, 2)
    )  # [bs, seq, ck, hw]
    v4 = np.ones((BS, SEQ, HW, 4), np.float32)
    v4[:, :, :, :CV] = v.reshape(BS, SEQ, HW, CV)
    # partition-major: [bs, 128, seq, 8, 4]
    v4h = np.ascontiguousarray(
        v4.reshape(BS, SEQ, NT, 128, 4).transpose(0, 3, 1, 2, 4).reshape(
            BS, 128, SEQ, NT * 4
        )
    )
    import ml_dtypes

    g = (1.0 / (1.0 + np.exp(-attention.reshape(BS, SEQ, HW)))).astype(np.float32)
    gflat = np.ascontiguousarray(
        g.astype(ml_dtypes.bfloat16).reshape(BS, 1, SEQ * HW)
    )
    g4h = np.ascontiguousarray(
        np.repeat(
            g.reshape(BS, SEQ, NT, 128)[:, :, :, :, None], 4, axis=4
        ).transpose(0, 3, 1, 2, 4).reshape(BS, 128, SEQ, NT * 4)
    )
    maskf = seq_mask.astype(np.float32)
    return kt, v4h, gflat, g4h, maskf


def kernel(k, v, attention, seq_mask):
    k = np.asarray(k, dtype=np.float32)
    v = np.asarray(v, dtype=np.float32)
    attention = np.asarray(attention, dtype=np.float32)
    seq_mask = np.asarray(seq_mask)

    if "nc" not in _CACHE:
        _CACHE["nc"] = build_program()
    nc = _CACHE["nc"]

    kt, v4h, gflat, g4h, maskf = _prep_inputs(k, v, attention, seq_mask)

    in_maps = []
    for b in range(BS):
        in_maps.append(
            {
                "kt": kt[b],
                "v4h": v4h[b],
                "gflat": gflat[b],
                "g4h": g4h[b],
                "maskf": np.ascontiguousarray(maskf[b : b + 1]),
            }
        )
    res = run_bass_kernel_spmd(nc, in_maps, list(range(BS)))
    out_v = np.stack([res.results[b]["out_v"] for b in range(BS)]).reshape(
        BS, SEQ - 1, H, W, CV
    )
    gt = v[:, 1:].reshape(BS, SEQ - 1, H, W, CV)
    return out_v, gt


# revision 7
# speedup vs baseline: 1.0894x; 1.0038x over previous
"""KAST scatter-memory kernel for Trainium2 (8 NeuronCores, data-parallel over batch).

Per core: one batch element, 15 sequential steps.

Host precomputes (numpy, inside kernel()):
  kt   = k transposed to [seq, ck, hw]   (no on-device PE transposes)
  g    = sigmoid(attention)              (no on-device sigmoid / ACT table swaps)
  v4   = [v | 1] ones-padded, partition-major  (denominator column ready)
  g4   = gate natural-layout replicated x4, partition-major

Device, per step i:
  m_kT = m_kT + G*(kT_i - m_kT)          (DVE, ping-pong buffers)
  L    = kT_i^T/m_kT^T @ kT_{i+1}        ([kk, q] fp32r matmuls)
  E    = exp(L - 60)                     (2048-wide ACT exps over psum slot pairs)
  rec  = E^T(stationary) @ [pv|1]/[mv|1] (tiny [128,4]-output matmuls, natural layout)
  rec  = 0.9*Nk/Dk + 0.1*Nm/Dm          (DVE reciprocal/blend)
  pv   = mask_i ? v_i : rec

PSUM is one persistent [128, 4, 1024] tile (all 8 banks) used as 4 rotating
fill slots; adjacent slot pairs are exp'd by single 2048-wide activations,
and the rec accumulators live in transient slot corners (subtile deps).
"""
import sys

sys.path.insert(0, "/opt/trn_rl_repo")

import numpy as np

import concourse.bass as bass
import concourse.tile as tile
from concourse import bacc, mybir
from concourse.bass_utils import run_bass_kernel_spmd

F32 = mybir.dt.float32
F32R = mybir.dt.float32r
BF16 = mybir.dt.bfloat16
AF = mybir.ActivationFunctionType

BS, SEQ, H, W, CK = 8, 16, 32, 32, 256
HW = H * W          # 1024
CV = 3
NT = HW // 128      # 8 hw tiles
NC2 = CK // 128     # 2 ck chunks
SHIFT = 60.0        # exp(logit - SHIFT); logits empirically <= 136, rowmax >= 23
COEF = 0.1

_CACHE = {}


def _r(x):
    return x.bitcast(F32R)


def build_program():
    nc = bacc.Bacc("TRN2", target_bir_lowering=False, debug=False, num_devices=8)

    kt_d = nc.dram_tensor("kt", [SEQ, CK, HW], F32R, kind="ExternalInput")
    v4_d = nc.dram_tensor("v4h", [128, SEQ, NT * 4], F32R, kind="ExternalInput")
    g_d = nc.dram_tensor("gflat", [1, SEQ * HW], BF16, kind="ExternalInput")
    g4_d = nc.dram_tensor("g4h", [128, SEQ, NT * 4], F32, kind="ExternalInput")
    m_d = nc.dram_tensor("maskf", [1, SEQ], F32, kind="ExternalInput")
    o_d = nc.dram_tensor("out_v", [SEQ - 1, HW, CV], F32, kind="ExternalOutput")

    with tile.TileContext(nc) as tc:
        with (
            tc.tile_pool(name="persist", bufs=1) as P1,
            tc.tile_pool(name="kt", bufs=4) as PKT,
            tc.tile_pool(name="tmp", bufs=2) as PT,
            tc.tile_pool(name="gb", bufs=3) as PG,
            tc.tile_pool(name="ek", bufs=5) as PEK,
            tc.tile_pool(name="em", bufs=5) as PEM,
            tc.tile_pool(name="small", bufs=3) as PSM,
            tc.tile_pool(name="psA", bufs=1, space="PSUM") as PSA,
        ):
            negC = P1.tile([128, 1], F32)
            nc.vector.memset(negC, -SHIFT)

            # All of PSUM: 4 rotating fill slots of [128, 1024] (2 banks each)
            ps_all = PSA.tile([128, 4, HW], F32, tag="psall")
            rot = [0]

            def load_kT_chunks(i):
                """kT frame as 2 chunk DMAs so first-chunk matmuls start early."""
                kT = PKT.tile([128, NC2, HW], F32R, tag="kT", name=f"kT{i}")
                for c in range(NC2):
                    nc.sync.dma_start(
                        out=kT[:, c, :],
                        in_=kt_d[i, c * 128 : (c + 1) * 128, :].rearrange(
                            "(o p) w -> p o w", p=128
                        ),
                    )
                return kT.rearrange("p c w -> p (c w)")

            def load_G(i):
                Gt = PG.tile([128, HW], BF16, tag="G", name=f"G{i}")
                nc.gpsimd.dma_start(
                    out=Gt, in_=g_d[0:1, i * HW : (i + 1) * HW].partition_broadcast(128)
                )
                return Gt

            # prologue: kT frames first -- the serial DMA stream must deliver
            # kT1 before anything bulky, since the first matmul waits on it
            kT_i = load_kT_chunks(0)
            kT_n = load_kT_chunks(1)
            G_rows = [load_G(0), load_G(1)]
            Vall = P1.tile([128, SEQ, NT * 4], F32R, tag="Vall")
            nc.sync.dma_start(out=Vall, in_=v4_d[:, :, :])
            G4all = P1.tile([128, SEQ, NT * 4], F32, tag="G4all")
            nc.sync.dma_start(out=G4all, in_=g4_d[:, :, :])
            Mall = P1.tile([128, SEQ], F32, tag="Mall")
            nc.gpsimd.dma_start(out=Mall, in_=m_d[0:1, :].partition_broadcast(128))
            pv1 = Vall[:, 0, :]

            # persistent state: m_kT ping-pong [128, (c w)] = [ck-part, 2 x hw]
            m_kT = [P1.tile([128, NC2 * HW], F32, tag=f"mkT{j}", name=f"mkT{j}") for j in range(2)]
            nc.vector.memset(m_kT[0], 0.0)
            mv1 = P1.tile([128, 4 * NT], F32, tag="mv1")
            nc.vector.memset(mv1, 0.0)
            nc.vector.memset(mv1[:, 3 : 4 * NT : 4], 1.0)

            def fill_slot(lhs, kTn, t):
                """4 c-outer fp32r matmuls of tile t into the next psum slot."""
                s = rot[0] % 4
                rot[0] += 1
                for c in range(NC2):
                    for half in range(2):
                        nc.tensor.matmul(
                            ps_all[:, s, half * 512 : (half + 1) * 512],
                            _r(lhs[:, c * HW + t * 128 : c * HW + (t + 1) * 128]),
                            _r(kTn[:, c * HW + half * 512 : c * HW + (half + 1) * 512]),
                            start=(c == 0),
                            stop=(c == NC2 - 1),
                        )
                return s

            for i in range(SEQ - 1):
                kT_n2 = load_kT_chunks(i + 2) if i + 2 <= SEQ - 1 else None
                if i + 2 <= SEQ - 2:
                    G_rows.append(load_G(i + 2))
                G = G_rows[i]
                gb32 = G4all[:, i, :]

                # --- m_kT EMA on DVE (ping-pong): mnew = mcur + G*(kT_i - mcur)
                mcur, mnew = m_kT[i % 2], m_kT[(i + 1) % 2]
                for c in range(NC2):
                    sl = slice(c * HW, (c + 1) * HW)
                    tmp = PT.tile([128, HW], F32, tag=f"tmpk{c}")
                    nc.vector.tensor_sub(tmp, kT_i[:, sl].bitcast(F32), mcur[:, sl])
                    nc.vector.tensor_mul(tmp, tmp, G)
                    nc.vector.tensor_add(_r(mnew[:, sl]), mcur[:, sl], tmp)

                # --- m_v EMA: mv1 += gb32 * (pv1 - mv1)  (ones col stays 1)
                tmpv = PSM.tile([128, 4 * NT], F32, tag="tmpv")
                nc.vector.tensor_sub(tmpv, pv1.bitcast(F32), mv1)
                nc.vector.tensor_mul(tmpv, tmpv, gb32)
                nc.vector.tensor_add(_r(mv1), mv1, tmpv)

                # --- logits + 2048-wide exp per psum slot pair, then rec, per sim
                Nk = Nm = None
                for lhs, pool, nm in ((kT_i, PEK, "ek"), (mnew, PEM, "em")):
                    E = []
                    for pj in range(4):
                        s0 = fill_slot(lhs, kT_n, 2 * pj)
                        s1 = fill_slot(lhs, kT_n, 2 * pj + 1)
                        assert s1 == s0 + 1 and s0 % 2 == 0, (s0, s1)
                        Ep = pool.tile([128, 2 * HW], F32, tag=nm, name=f"{nm}{i}_{pj}")
                        nc.scalar.activation(
                            _r(Ep),
                            ps_all[:, s0 : s0 + 2, :].rearrange("p s w -> p (s w)"),
                            AF.Exp,
                            bias=negC[:, 0:1],
                        )
                        E.append(Ep)

                    # rec for this sim: corner of the slot refilled the latest
                    # (subtile deps order the next fill after the Nhalf copy)
                    rhs1 = pv1 if nm == "ek" else mv1
                    cs = (rot[0] + 3) % 4
                    base = 0 if nm == "ek" else 32
                    psN = ps_all[:, cs, base : base + 32]
                    for q in range(NT):
                        out_sl = psN[:, q * 4 : (q + 1) * 4]
                        for c in range(NT):
                            nc.tensor.matmul(
                                out_sl,
                                _r(E[c // 2][:, (c % 2) * HW + q * 128 : (c % 2) * HW + (q + 1) * 128]),
                                _r(rhs1[:, c * 4 : (c + 1) * 4]),
                                start=(c == 0),
                                stop=(c == NT - 1),
                            )
                    Nhalf = PSM.tile([128, 32], F32, tag=f"N{nm}", name=f"N{nm}{i}")
                    nc.vector.tensor_copy(out=Nhalf, in_=psN)
                    if nm == "ek":
                        Nk = Nhalf
                    else:
                        Nm = Nhalf

                # --- rec = 0.9*Nk/Dk + 0.1*Nm/Dm
                rD = PSM.tile([128, 16], F32, tag="rD")
                nc.vector.reciprocal(rD[:, 0:8], Nk[:, 3:32:4])
                nc.vector.reciprocal(rD[:, 8:16], Nm[:, 3:32:4])
                nc.vector.tensor_scalar_mul(rD[:, 0:8], rD[:, 0:8], 1.0 - COEF)
                nc.vector.tensor_scalar_mul(rD[:, 8:16], rD[:, 8:16], COEF)
                rDe = PSM.tile([128, 16, 4], F32, tag="rDe")
                nc.vector.tensor_copy(
                    out=rDe, in_=rD.unsqueeze(-1).broadcast_to([128, 16, 4])
                )
                rDe = rDe.rearrange("p t c -> p (t c)")
                Nsk = PSM.tile([128, 32], F32, tag="Nsk")
                nc.vector.tensor_mul(Nsk, Nk, rDe[:, 0:32])
                rec = PSM.tile([128, 32], F32, tag="rec")
                nc.vector.tensor_mul(rec, Nm, rDe[:, 32:64])
                nc.vector.tensor_add(rec, rec, Nsk)

                # --- write out_v[i] (pre-blend reconstruction)
                nc.sync.dma_start(
                    out=o_d[i].rearrange("(t p) c -> p t c", p=128),
                    in_=rec.rearrange("p (t c) -> p t c", c=4)[:, :, 0:CV],
                )

                # --- pv_next = rec + mask_i * (v1_i - rec)
                if i < SEQ - 2:
                    v1 = Vall[:, i, :]
                    diff = PSM.tile([128, 32], F32, tag="diff")
                    nc.vector.tensor_sub(diff, v1.bitcast(F32), rec)
                    nc.vector.tensor_scalar_mul(diff, diff, Mall[:, i : i + 1])
                    pv1_new = PSM.tile([128, 32], F32, tag="pv1")
                    nc.vector.tensor_add(_r(pv1_new), rec, diff)
                    pv1 = pv1_new
                    kT_i = kT_n
                    kT_n = kT_n2

    nc.compile()
    return nc


def _prep_inputs(k, v, attention, seq_mask):
    import ml_dtypes

    kt = np.ascontiguousarray(
        k.reshape(BS, SEQ, HW, CK).transpose(0, 1, 3, 2)
    )  # [bs, seq, ck, hw]
    v4 = np.ones((BS, SEQ, HW, 4), np.float32)
    v4[:, :, :, :CV] = v.reshape(BS, SEQ, HW, CV)
    # partition-major: [bs, 128, seq, 8*4]
    v4h = np.ascontiguousarray(
        v4.reshape(BS, SEQ, NT, 128, 4).transpose(0, 3, 1, 2, 4).reshape(
            BS, 128, SEQ, NT * 4
        )
    )
    g = (1.0 / (1.0 + np.exp(-attention.reshape(BS, SEQ, HW)))).astype(np.float32)
    gflat = np.ascontiguousarray(
        g.astype(ml_dtypes.bfloat16).reshape(BS, 1, SEQ * HW)
    )
    g4h = np.ascontiguousarray(
        np.repeat(
            g.reshape(BS, SEQ, NT, 128)[:, :, :, :, None], 4, axis=4
        ).transpose(0, 3, 1, 2, 4).reshape(BS, 128, SEQ, NT * 4)
    )
    maskf = seq_mask.astype(np.float32)
    return kt, v4h, gflat, g4h, maskf


def kernel(k, v, attention, seq_mask):
    k = np.asarray(k, dtype=np.float32)
    v = np.asarray(v, dtype=np.float32)
    attention = np.asarray(attention, dtype=np.float32)
    seq_mask = np.asarray(seq_mask)

    if "nc" not in _CACHE:
        _CACHE["nc"] = build_program()
    nc = _CACHE["nc"]

    kt, v4h, gflat, g4h, maskf = _prep_inputs(k, v, attention, seq_mask)

    in_maps = []
    for b in range(BS):
        in_maps.append(
            {
                "kt": kt[b],
                "v4h": v4h[b],
                "gflat": gflat[b],
                "g4h": g4h[b],
                "maskf": np.ascontiguousarray(maskf[b : b + 1]),
            }
        )
    res = run_bass_kernel_spmd(nc, in_maps, list(range(BS)))
    out_v = np.stack([res.results[b]["out_v"] for b in range(BS)]).reshape(
        BS, SEQ - 1, H, W, CV
    )
    gt = v[:, 1:].reshape(BS, SEQ - 1, H, W, CV)
    return out_v, gt


# revision 8
# speedup vs baseline: 1.0974x; 1.0074x over previous
"""KAST scatter-memory kernel for Trainium2 (8 NeuronCores, data-parallel over batch).

Per core: one batch element, 15 sequential steps.

Host precomputes (numpy, inside kernel()):
  kt   = k transposed to [seq, ck, hw]   (no on-device PE transposes)
  g    = sigmoid(attention)              (no on-device sigmoid / ACT table swaps)
  v4   = [v | 1] ones-padded, partition-major  (denominator column ready)
  g4   = gate natural-layout replicated x4, partition-major

Device, per step i:
  m_kT = m_kT + G*(kT_i - m_kT)          (DVE, ping-pong buffers)
  L    = kT_i^T/m_kT^T @ kT_{i+1}        ([kk, q] fp32r matmuls)
  E    = exp(L - 60)                     (2048-wide ACT exps over psum slot pairs)
  rec  = E^T(stationary) @ [pv|1]/[mv|1] (tiny [128,4]-output matmuls, natural layout)
  rec  = 0.9*Nk/Dk + 0.1*Nm/Dm          (DVE reciprocal/blend)
  pv   = mask_i ? v_i : rec

PSUM is one persistent [128, 4, 1024] tile (all 8 banks) used as 4 rotating
fill slots; adjacent slot pairs are exp'd by single 2048-wide activations,
and the rec accumulators live in transient slot corners (subtile deps).
"""
import sys

sys.path.insert(0, "/opt/trn_rl_repo")

import numpy as np

import concourse.bass as bass
import concourse.tile as tile
from concourse import bacc, mybir
from concourse.bass_utils import run_bass_kernel_spmd

F32 = mybir.dt.float32
F32R = mybir.dt.float32r
BF16 = mybir.dt.bfloat16
AF = mybir.ActivationFunctionType

BS, SEQ, H, W, CK = 8, 16, 32, 32, 256
HW = H * W          # 1024
CV = 3
NT = HW // 128      # 8 hw tiles
NC2 = CK // 128     # 2 ck chunks
SHIFT = 60.0        # exp(logit - SHIFT); logits empirically <= 136, rowmax >= 23
COEF = 0.1

_CACHE = {}


def _r(x):
    return x.bitcast(F32R)


def build_program():
    nc = bacc.Bacc("TRN2", target_bir_lowering=False, debug=False, num_devices=8)

    kt_d = nc.dram_tensor("kt", [SEQ, CK, HW], F32R, kind="ExternalInput")
    v4_d = nc.dram_tensor("v4h", [128, SEQ, NT * 4], F32R, kind="ExternalInput")
    g_d = nc.dram_tensor("gflat", [1, SEQ * HW], BF16, kind="ExternalInput")
    g4_d = nc.dram_tensor("g4h", [128, SEQ, NT * 4], F32, kind="ExternalInput")
    m_d = nc.dram_tensor("maskf", [1, SEQ], F32, kind="ExternalInput")
    o_d = nc.dram_tensor("out_v", [SEQ - 1, HW, CV], F32, kind="ExternalOutput")

    with tile.TileContext(nc) as tc:
        with (
            tc.tile_pool(name="persist", bufs=1) as P1,
            tc.tile_pool(name="kt", bufs=4) as PKT,
            tc.tile_pool(name="tmp", bufs=2) as PT,
            tc.tile_pool(name="gb", bufs=3) as PG,
            tc.tile_pool(name="ek", bufs=5) as PEK,
            tc.tile_pool(name="em", bufs=5) as PEM,
            tc.tile_pool(name="small", bufs=3) as PSM,
            tc.tile_pool(name="psA", bufs=1, space="PSUM") as PSA,
        ):
            negC = P1.tile([128, 1], F32)
            nc.vector.memset(negC, -SHIFT)

            # All of PSUM: 4 rotating fill slots of [128, 1024] (2 banks each)
            ps_all = PSA.tile([128, 4, HW], F32, tag="psall")
            rot = [0]

            def load_kT_chunks(i):
                """kT frame as 2 chunk DMAs so first-chunk matmuls start early."""
                kT = PKT.tile([128, NC2, HW], F32R, tag="kT", name=f"kT{i}")
                for c in range(NC2):
                    nc.sync.dma_start(
                        out=kT[:, c, :],
                        in_=kt_d[i, c * 128 : (c + 1) * 128, :].rearrange(
                            "(o p) w -> p o w", p=128
                        ),
                    )
                return kT.rearrange("p c w -> p (c w)")

            def load_G(i):
                Gt = PG.tile([128, HW], BF16, tag="G", name=f"G{i}")
                nc.gpsimd.dma_start(
                    out=Gt, in_=g_d[0:1, i * HW : (i + 1) * HW].partition_broadcast(128)
                )
                return Gt

            # prologue: kT frames first, c0 chunks of BOTH frames before c1
            # chunks (first matmuls need only c0), then everything bulky
            kT01 = [
                PKT.tile([128, NC2, HW], F32R, tag="kT", name=f"kT{i}")
                for i in range(2)
            ]
            for c in range(NC2):
                for i in range(2):
                    nc.sync.dma_start(
                        out=kT01[i][:, c, :],
                        in_=kt_d[i, c * 128 : (c + 1) * 128, :].rearrange(
                            "(o p) w -> p o w", p=128
                        ),
                    )
            kT_i = kT01[0].rearrange("p c w -> p (c w)")
            kT_n = kT01[1].rearrange("p c w -> p (c w)")
            G_rows = [load_G(0), load_G(1)]
            # PE ramp warm-up: garbage matmuls into slot 3 (overwritten later)
            Wm = P1.tile([128, 512], F32, tag="Wm")
            nc.vector.memset(Wm, 0.0)
            nc.vector.tensor_copy(out=_r(Wm), in_=Wm)
            for _ in range(8):
                nc.tensor.matmul(
                    ps_all[:, 3, 0:512], _r(Wm[:, 0:128]), _r(Wm),
                    start=True, stop=True,
                )
            Vall = P1.tile([128, SEQ, NT * 4], F32R, tag="Vall")
            nc.sync.dma_start(out=Vall, in_=v4_d[:, :, :])
            G4all = P1.tile([128, SEQ, NT * 4], F32, tag="G4all")
            nc.sync.dma_start(out=G4all, in_=g4_d[:, :, :])
            Mall = P1.tile([128, SEQ], F32, tag="Mall")
            nc.gpsimd.dma_start(out=Mall, in_=m_d[0:1, :].partition_broadcast(128))
            pv1 = Vall[:, 0, :]

            # persistent state: m_kT ping-pong [128, (c w)] = [ck-part, 2 x hw]
            m_kT = [P1.tile([128, NC2 * HW], F32, tag=f"mkT{j}", name=f"mkT{j}") for j in range(2)]
            nc.vector.memset(m_kT[0], 0.0)
            mv1 = P1.tile([128, 4 * NT], F32, tag="mv1")
            nc.vector.memset(mv1, 0.0)
            nc.vector.memset(mv1[:, 3 : 4 * NT : 4], 1.0)

            def ema_mkT(i, kTfrm):
                mcur, mnew = m_kT[i % 2], m_kT[(i + 1) % 2]
                for c in range(NC2):
                    sl = slice(c * HW, (c + 1) * HW)
                    tmp = PT.tile([128, HW], F32, tag=f"tmpk{c}")
                    nc.vector.tensor_sub(tmp, kTfrm[:, sl].bitcast(F32), mcur[:, sl])
                    nc.vector.tensor_mul(tmp, tmp, G_rows[i])
                    nc.vector.tensor_add(_r(mnew[:, sl]), mcur[:, sl], tmp)
                return mnew

            def fill_slot(lhs, kTn, t):
                """4 c-outer fp32r matmuls of tile t into the next psum slot."""
                s = rot[0] % 4
                rot[0] += 1
                for c in range(NC2):
                    for half in range(2):
                        nc.tensor.matmul(
                            ps_all[:, s, half * 512 : (half + 1) * 512],
                            _r(lhs[:, c * HW + t * 128 : c * HW + (t + 1) * 128]),
                            _r(kTn[:, c * HW + half * 512 : c * HW + (half + 1) * 512]),
                            start=(c == 0),
                            stop=(c == NC2 - 1),
                        )
                return s

            mnew = ema_mkT(0, kT_i)
            for i in range(SEQ - 1):
                kT_n2 = load_kT_chunks(i + 2) if i + 2 <= SEQ - 1 else None
                if i + 2 <= SEQ - 2:
                    G_rows.append(load_G(i + 2))
                gb32 = G4all[:, i, :]

                # --- m_v EMA: mv1 += gb32 * (pv1 - mv1)  (ones col stays 1)
                tmpv = PSM.tile([128, 4 * NT], F32, tag="tmpv")
                nc.vector.tensor_sub(tmpv, pv1.bitcast(F32), mv1)
                nc.vector.tensor_mul(tmpv, tmpv, gb32)
                nc.vector.tensor_add(_r(mv1), mv1, tmpv)

                # --- logits + 2048-wide exp per psum slot pair, then rec, per sim
                Nk = Nm = None
                for lhs, pool, nm in ((kT_i, PEK, "ek"), (mnew, PEM, "em")):
                    E = []
                    for pj in range(4):
                        s0 = fill_slot(lhs, kT_n, 2 * pj)
                        s1 = fill_slot(lhs, kT_n, 2 * pj + 1)
                        assert s1 == s0 + 1 and s0 % 2 == 0, (s0, s1)
                        Ep = pool.tile([128, 2 * HW], F32, tag=nm, name=f"{nm}{i}_{pj}")
                        nc.scalar.activation(
                            _r(Ep),
                            ps_all[:, s0 : s0 + 2, :].rearrange("p s w -> p (s w)"),
                            AF.Exp,
                            bias=negC[:, 0:1],
                        )
                        E.append(Ep)

                    # rec for this sim: corner of the slot refilled the latest
                    # (subtile deps order the next fill after the Nhalf copy)
                    rhs1 = pv1 if nm == "ek" else mv1
                    cs = (rot[0] + 3) % 4
                    base = 0 if nm == "ek" else 32
                    psN = ps_all[:, cs, base : base + 32]
                    for q in range(NT):
                        out_sl = psN[:, q * 4 : (q + 1) * 4]
                        for c in range(NT):
                            nc.tensor.matmul(
                                out_sl,
                                _r(E[c // 2][:, (c % 2) * HW + q * 128 : (c % 2) * HW + (q + 1) * 128]),
                                _r(rhs1[:, c * 4 : (c + 1) * 4]),
                                start=(c == 0),
                                stop=(c == NT - 1),
                            )
                    Nhalf = PSM.tile([128, 32], F32, tag=f"N{nm}", name=f"N{nm}{i}")
                    nc.vector.tensor_copy(out=Nhalf, in_=psN)
                    if nm == "ek":
                        Nk = Nhalf
                    else:
                        Nm = Nhalf

                # --- EMA for the next step, issued now so the DVE FIFO
                # completes it long before step i+1's E_m fills need it
                if i + 1 <= SEQ - 2:
                    mnext = ema_mkT(i + 1, kT_n)

                # --- rec = 0.9*Nk/Dk + 0.1*Nm/Dm
                rD = PSM.tile([128, 16], F32, tag="rD")
                nc.vector.reciprocal(rD[:, 0:8], Nk[:, 3:32:4])
                nc.vector.reciprocal(rD[:, 8:16], Nm[:, 3:32:4])
                nc.vector.tensor_scalar_mul(rD[:, 0:8], rD[:, 0:8], 1.0 - COEF)
                nc.vector.tensor_scalar_mul(rD[:, 8:16], rD[:, 8:16], COEF)
                rDe = PSM.tile([128, 16, 4], F32, tag="rDe")
                nc.vector.tensor_copy(
                    out=rDe, in_=rD.unsqueeze(-1).broadcast_to([128, 16, 4])
                )
                rDe = rDe.rearrange("p t c -> p (t c)")
                Nsk = PSM.tile([128, 32], F32, tag="Nsk")
                nc.vector.tensor_mul(Nsk, Nk, rDe[:, 0:32])
                rec = PSM.tile([128, 32], F32, tag="rec")
                nc.vector.tensor_mul(rec, Nm, rDe[:, 32:64])
                nc.vector.tensor_add(rec, rec, Nsk)

                # --- write out_v[i] (pre-blend reconstruction)
                nc.sync.dma_start(
                    out=o_d[i].rearrange("(t p) c -> p t c", p=128),
                    in_=rec.rearrange("p (t c) -> p t c", c=4)[:, :, 0:CV],
                )

                # --- pv_next = rec + mask_i * (v1_i - rec)
                if i < SEQ - 2:
                    v1 = Vall[:, i, :]
                    diff = PSM.tile([128, 32], F32, tag="diff")
                    nc.vector.tensor_sub(diff, v1.bitcast(F32), rec)
                    nc.vector.tensor_scalar_mul(diff, diff, Mall[:, i : i + 1])
                    pv1_new = PSM.tile([128, 32], F32, tag="pv1")
                    nc.vector.tensor_add(_r(pv1_new), rec, diff)
                    pv1 = pv1_new
                    kT_i = kT_n
                    kT_n = kT_n2
                    mnew = mnext

    nc.compile()
    return nc


def _prep_inputs(k, v, attention, seq_mask):
    import ml_dtypes

    kt = np.ascontiguousarray(
        k.reshape(BS, SEQ, HW, CK).transpose(0, 1, 3, 2)
    )  # [bs, seq, ck, hw]
    v4 = np.ones((BS, SEQ, HW, 4), np.float32)
    v4[:, :, :, :CV] = v.reshape(BS, SEQ, HW, CV)
    # partition-major: [bs, 128, seq, 8*4]
    v4h = np.ascontiguousarray(
        v4.reshape(BS, SEQ, NT, 128, 4).transpose(0, 3, 1, 2, 4).reshape(
            BS, 128, SEQ, NT * 4
        )
    )
    g = (1.0 / (1.0 + np.exp(-attention.reshape(BS, SEQ, HW)))).astype(np.float32)
    gflat = np.ascontiguousarray(
        g.astype(ml_dtypes.bfloat16).reshape(BS, 1, SEQ * HW)
    )
    g4h = np.ascontiguousarray(
        np.repeat(
            g.reshape(BS, SEQ, NT, 128)[:, :, :, :, None], 4, axis=4
        ).transpose(0, 3, 1, 2, 4).reshape(BS, 128, SEQ, NT * 4)
    )
    maskf = seq_mask.astype(np.float32)
    return kt, v4h, gflat, g4h, maskf


def kernel(k, v, attention, seq_mask):
    k = np.asarray(k, dtype=np.float32)
    v = np.asarray(v, dtype=np.float32)
    attention = np.asarray(attention, dtype=np.float32)
    seq_mask = np.asarray(seq_mask)

    if "nc" not in _CACHE:
        _CACHE["nc"] = build_program()
    nc = _CACHE["nc"]

    kt, v4h, gflat, g4h, maskf = _prep_inputs(k, v, attention, seq_mask)

    in_maps = []
    for b in range(BS):
        in_maps.append(
            {
                "kt": kt[b],
                "v4h": v4h[b],
                "gflat": gflat[b],
                "g4h": g4h[b],
                "maskf": np.ascontiguousarray(maskf[b : b + 1]),
            }
        )
    res = run_bass_kernel_spmd(nc, in_maps, list(range(BS)))
    out_v = np.stack([res.results[b]["out_v"] for b in range(BS)]).reshape(
        BS, SEQ - 1, H, W, CV
    )
    gt = v[:, 1:].reshape(BS, SEQ - 1, H, W, CV)
    return out_v, gt
